# revision 3
# baseline (speedup 1.0000x reference)
"""Trainium2 Bass kernel for nn_DenseControl_81793357185325.

Math: out = broadcast_to(s, [N, D]) where
    s = sum_{n: rank[n] <= 100} (x[n] @ W.T + b)
      = (sum_masked x[n]) @ W.T + count * b

Device strategy (8 NeuronCores, data-parallel over rows):
  - pad N=100000 -> 100352 = 8 * 12544 = 8 * 128 * 98, shard rows
  - per core: stream x shard [12544, 256] through SBUF; one matmul per
    98-row "slot" accumulates the mask-weighted partition-sum into PSUM
    (lhsT = mask column [128,1], rhs = x slot [128,256])
  - partial s_i = xsum_i @ W.T + count_i * b computed on-device (PE matvec)
  - AllReduce [256] across the 8 cores (collective, DRAM bounce)
  - broadcast s to a [128, 8*256] SBUF tile (K=1 matmuls) and stream the
    output shard [12544, 256] back to DRAM
"""

import numpy as np

import concourse.bass as bass
import concourse.tile as tile
from concourse import bacc, mybir
from concourse.bass_utils import run_bass_kernel_spmd

N_CORES = 8
N = 100000
D = 256
K_THRESH = 100.0

P = 128                    # SBUF partitions
SLOTS = 98                 # rows per partition per core
SHARD = P * SLOTS          # 12544 rows per core
NPAD = SHARD * N_CORES     # 100352

CHUNK_SLOTS = 7            # slots per x DMA (7 KiB contiguous per partition)
N_CHUNKS = SLOTS // CHUNK_SLOTS   # 14 read DMAs per core

BC_SLOTS = 8               # copies of s in the broadcast tile
WCHUNK_SLOTS = 7           # slots per output DMA -> 14 write DMAs

_compiled = None


def _build():
    nc = bacc.Bacc(
        "TRN2", target_bir_lowering=False, debug=False, num_devices=N_CORES
    )

    x = nc.dram_tensor("x", [SHARD, D], mybir.dt.float32, kind="ExternalInput")
    rk = nc.dram_tensor("rk", [SHARD], mybir.dt.float32, kind="ExternalInput")
    wt = nc.dram_tensor("wt", [D, D], mybir.dt.float32, kind="ExternalInput")
    bv = nc.dram_tensor("bv", [D], mybir.dt.float32, kind="ExternalInput")
    out = nc.dram_tensor("out", [SHARD, D], mybir.dt.float32, kind="ExternalOutput")

    x_r = x.ap().rearrange("(p n) d -> p n d", p=P)      # [128, 98, 256]
    out_r = out.ap().rearrange("(p n) d -> p n d", p=P)  # [128, 98, 256]

    with tile.TileContext(nc) as tc:
        with (
            tc.tile_pool(name="xin", bufs=4) as xin,
            tc.tile_pool(name="consts", bufs=1) as consts,
            tc.tile_pool(name="small", bufs=1) as small,
            tc.tile_pool(name="psum_acc", bufs=1, space="PSUM") as psum_acc,
            tc.tile_pool(name="psum_bc", bufs=1, space="PSUM") as psum_bc_pool,
            tc.tile_pool(name="dram", bufs=1, space="DRAM") as dram,
        ):
            # ---- constants / small tiles ----
            mask = consts.tile([P, SLOTS], mybir.dt.float32)
            rk_tile = consts.tile([P, SLOTS], mybir.dt.float32)
            nc.sync.dma_start(rk_tile[:], rk.ap().rearrange("(p n) -> p n", p=P))
            nc.vector.tensor_scalar(
                mask[:], rk_tile[:], K_THRESH, None, mybir.AluOpType.is_le
            )

            ones_col = consts.tile([P, 1], mybir.dt.float32)
            nc.vector.memset(ones_col[:], 1.0)
            ones_row = consts.tile([1, P], mybir.dt.float32)
            nc.vector.memset(ones_row[:], 1.0)

            wt_sb = consts.tile([P, 2 * D], mybir.dt.float32)  # [128, 2*256]
            for k in range(2):
                nc.sync.dma_start(
                    wt_sb[:, k * D : (k + 1) * D], wt[k * P : (k + 1) * P, :]
                )
            b_sb = consts.tile([1, D], mybir.dt.float32)
            nc.sync.dma_start(b_sb[:], bv.ap().rearrange("(o d) -> o d", o=1))

            # ---- phase 1: masked partition-sum of x over the shard ----
            psum_s = psum_acc.tile([1, D], mybir.dt.float32)
            for c in range(N_CHUNKS):
                xt = xin.tile([P, CHUNK_SLOTS * D], mybir.dt.float32)
                nc.sync.dma_start(
                    xt[:].rearrange("p (n d) -> p n d", d=D),
                    x_r[:, c * CHUNK_SLOTS : (c + 1) * CHUNK_SLOTS, :],
                )
                for s in range(CHUNK_SLOTS):
                    col = c * CHUNK_SLOTS + s
                    nc.tensor.matmul(
                        psum_s[:],
                        mask[:, col : col + 1],
                        xt[:, s * D : (s + 1) * D],
                        start=(col == 0),
                        stop=(col == SLOTS - 1),
                    )

            # count of masked rows in this shard -> PSUM [1,1]
            rowsum = small.tile([P, 1], mybir.dt.float32)
            nc.vector.reduce_sum(rowsum[:], mask[:], axis=mybir.AxisListType.X)
            psum_c = psum_acc.tile([1, 1], mybir.dt.float32)
            nc.tensor.matmul(
                psum_c[:], rowsum[:], ones_col[:], start=True, stop=True
            )

            # ---- phase 2: partial s_i = xsum_i @ W.T + count_i * b ----
            # transpose xsum [1,256] to [128,2] via DRAM round-trip
            xsum_sb = small.tile([1, D], mybir.dt.float32)
            nc.vector.tensor_copy(xsum_sb[:], psum_s[:])
            xsum_dram = dram.tile([D], mybir.dt.float32)
            nc.sync.dma_start(xsum_dram[:].rearrange("(o d) -> o d", o=1), xsum_sb[:])
            xsum_t = small.tile([P, 2], mybir.dt.float32)
            nc.sync.dma_start(
                xsum_t[:], xsum_dram[:].rearrange("(k p) -> p k", p=P)
            )

            psum_sv = psum_acc.tile([1, D], mybir.dt.float32)
            for k in range(2):
                nc.tensor.matmul(
                    psum_sv[:],
                    xsum_t[:, k : k + 1],
                    wt_sb[:, k * D : (k + 1) * D],
                    start=(k == 0),
                    stop=(k == 1),
                )

            cnt_sb = small.tile([1, 1], mybir.dt.float32)
            nc.vector.tensor_copy(cnt_sb[:], psum_c[:])
            cb = small.tile([1, D], mybir.dt.float32)
            nc.vector.tensor_scalar(
                cb[:], b_sb[:], cnt_sb[:], None, mybir.AluOpType.mult
            )
            s_part = small.tile([1, D], mybir.dt.float32)
            nc.vector.tensor_add(s_part[:], psum_sv[:], cb[:])

            # ---- phase 3: AllReduce the partial s across 8 cores ----
            cc_in = dram.tile([D], mybir.dt.float32)
            cc_out = dram.tile([D], mybir.dt.float32)
            nc.sync.dma_start(cc_in[:].rearrange("(o d) -> o d", o=1), s_part[:])
            nc.gpsimd.collective_compute(
                "AllReduce",
                mybir.AluOpType.add,
                replica_groups=[list(range(N_CORES))],
                ins=[cc_in.opt()],
                outs=[cc_out.opt()],
            )
            s_sb = small.tile([1, D], mybir.dt.float32)
            nc.sync.dma_start(s_sb[:], cc_out[:].rearrange("(o d) -> o d", o=1))

            # ---- phase 4: broadcast s to [128, BC_SLOTS*256] and stream out ----
            psum_bc = psum_bc_pool.tile([P, BC_SLOTS * D], mybir.dt.float32)
            for m in range(BC_SLOTS):
                nc.tensor.matmul(
                    psum_bc[:, m * D : (m + 1) * D],
                    ones_row[:],
                    s_sb[:],
                    start=True,
                    stop=True,
                )
            bc_sb = consts.tile([P, BC_SLOTS * D], mybir.dt.float32)
            nc.vector.tensor_copy(bc_sb[:], psum_bc[:])

            bc_v = bc_sb[:].rearrange("p (n d) -> p n d", d=D)
            for w in range(0, SLOTS, WCHUNK_SLOTS):
                ns = min(WCHUNK_SLOTS, SLOTS - w)
                nc.sync.dma_start(
                    out_r[:, w : w + ns, :], bc_v[:, :ns, :]
                )

    nc.compile()
    return nc


def _get_compiled():
    global _compiled
    if _compiled is None:
        _compiled = _build()
    return _compiled


def kernel(x, edge_index, node_rankings, W, b):
    x = np.asarray(x, dtype=np.float32)
    W = np.asarray(W, dtype=np.float32)
    b = np.asarray(b, dtype=np.float32)

    xp = np.zeros((NPAD, D), dtype=np.float32)
    xp[:N] = x
    # rankings as f32 (exact for values < 2^24); pad rows get +inf -> mask 0
    rkp = np.full((NPAD,), np.inf, dtype=np.float32)
    rkp[:N] = np.asarray(node_rankings).astype(np.float32)
    wt = np.ascontiguousarray(W.T)

    in_maps = [
        {
            "x": np.ascontiguousarray(xp[i * SHARD : (i + 1) * SHARD]),
            "rk": np.ascontiguousarray(rkp[i * SHARD : (i + 1) * SHARD]),
            "wt": wt,
            "bv": b,
        }
        for i in range(N_CORES)
    ]

    nc = _get_compiled()
    res = run_bass_kernel_spmd(nc, in_maps, core_ids=list(range(N_CORES)))
    full = np.concatenate([res.results[i]["out"] for i in range(N_CORES)], axis=0)
    return np.ascontiguousarray(full[:N])


# revision 8
# speedup vs baseline: 1.0183x; 1.0183x over previous
"""Trainium2 Bass kernel for nn_DenseControl_81793357185325.

Math: out = broadcast_to(s, [N, D]) where
    s = sum_{n: rank[n] <= 100} (x[n] @ W.T + b)
      = (sum_masked x[n]) @ W.T + count * b

Device strategy (8 NeuronCores, data-parallel over rows):
  - pad N=100000 -> 100352 = 8 * 12544 = 8 * 128 * 98, shard rows;
    partition p of a core holds its shard's rows [p*98, (p+1)*98)
  - stream x shard [12544, 256] through SBUF in 14 chunks; per 128-row
    "slot", accumulate mask-weighted sums: even slots on PE (psum
    accumulate, lhsT = mask column), odd slots on DVE
    (scalar_tensor_tensor multiply-add into an SBUF accumulator) so
    neither engine is the bottleneck and the phase is DMA-bound
  - count of masked rows rides as column 256 of the accumulator; one
    ones-matmul partition-sums the DVE accumulator into the same PSUM
  - AllGather the per-core partial [257] across 8 cores; each core
    sums the 8 partials locally (strided load puts xsum straight into
    the [128, 2] matvec layout), then s = xsum @ W.T + count * b
  - broadcast s to a [128, 8*256] SBUF tile and stream the output
    shard [12544, 256] back to DRAM
"""

import numpy as np

import concourse.bass as bass
import concourse.tile as tile
from concourse import bacc, mybir
from concourse.bass_utils import run_bass_kernel_spmd

N_CORES = 8
N = 100000
D = 256
K_THRESH = 100.0

P = 128                    # SBUF partitions
SLOTS = 98                 # rows per partition per core
SHARD = P * SLOTS          # 12544 rows per core
NPAD = SHARD * N_CORES     # 100352

CHUNK_SLOTS = 7            # slots per x DMA (7 KiB contiguous per partition)
N_CHUNKS = SLOTS // CHUNK_SLOTS   # 14 read DMAs per core

CC = D + 1                 # collective payload: [xsum(256) | count]

BC_SLOTS = 8               # copies of s in the broadcast tile
WCHUNK_SLOTS = 7           # slots per output DMA -> 14 write DMAs

_compiled = None


def _build():
    nc = bacc.Bacc(
        "TRN2", target_bir_lowering=False, debug=False, num_devices=N_CORES
    )

    x = nc.dram_tensor("x", [SHARD, D], mybir.dt.float32, kind="ExternalInput")
    rk = nc.dram_tensor("rk", [SHARD], mybir.dt.float32, kind="ExternalInput")
    wt = nc.dram_tensor("wt", [D, D], mybir.dt.float32, kind="ExternalInput")
    bv = nc.dram_tensor("bv", [D], mybir.dt.float32, kind="ExternalInput")
    out = nc.dram_tensor("out", [SHARD, D], mybir.dt.float32, kind="ExternalOutput")

    x_r = x.ap().rearrange("(p n) d -> p n d", p=P)      # [128, 98, 256]
    out_r = out.ap().rearrange("(p n) d -> p n d", p=P)  # [128, 98, 256]

    f32 = mybir.dt.float32

    with tile.TileContext(nc) as tc:
        with (
            tc.tile_pool(name="xin", bufs=6) as xin,
            tc.tile_pool(name="consts", bufs=1) as consts,
            tc.tile_pool(name="small", bufs=1) as small,
            tc.tile_pool(name="psum_acc", bufs=1, space="PSUM") as psum_acc,
            tc.tile_pool(name="psum_bc", bufs=1, space="PSUM") as psum_bc_pool,
            tc.tile_pool(name="dram", bufs=1, space="DRAM") as dram,
        ):
            # ---- constants / small tiles ----
            mask = consts.tile([P, SLOTS], f32)
            rk_tile = consts.tile([P, SLOTS], f32)
            nc.sync.dma_start(rk_tile[:], rk.ap().rearrange("(p n) -> p n", p=P))
            nc.vector.tensor_scalar(
                mask[:], rk_tile[:], K_THRESH, None, mybir.AluOpType.is_le
            )

            ones_col = consts.tile([P, 1], f32)
            nc.vector.memset(ones_col[:], 1.0)
            ones_row = consts.tile([1, P], f32)
            nc.vector.memset(ones_row[:], 1.0)

            wt_sb = consts.tile([P, 2 * D], f32)  # [128, 2*256]
            for k in range(2):
                nc.sync.dma_start(
                    wt_sb[:, k * D : (k + 1) * D], wt[k * P : (k + 1) * P, :]
                )
            b_sb = consts.tile([1, D], f32)
            nc.sync.dma_start(b_sb[:], bv.ap().rearrange("(o d) -> o d", o=1))

            # DVE-side accumulator; col 256 holds the mask row-sums (count)
            acc_v = consts.tile([P, CC], f32)
            nc.vector.memset(acc_v[:], 0.0)
            nc.vector.reduce_sum(
                acc_v[:, D : D + 1], mask[:], axis=mybir.AxisListType.X
            )

            # ---- phase 1: masked partition-sum of x over the shard ----
            # even slots -> PE psum accumulation, odd slots -> DVE mul-add
            psum_s = psum_acc.tile([1, D], f32)
            psum_v = psum_acc.tile([1, CC], f32)
            for c in range(N_CHUNKS):
                xt = xin.tile([P, CHUNK_SLOTS * D], f32)
                nc.sync.dma_start(
                    xt[:].rearrange("p (n d) -> p n d", d=D),
                    x_r[:, c * CHUNK_SLOTS : (c + 1) * CHUNK_SLOTS, :],
                )
                for s in range(CHUNK_SLOTS):
                    col = c * CHUNK_SLOTS + s
                    slot = xt[:, s * D : (s + 1) * D]
                    if col % 2 == 0:
                        nc.tensor.matmul(
                            psum_s[:],
                            mask[:, col : col + 1],
                            slot,
                            start=(col == 0),
                            stop=(col == SLOTS - 2),
                        )
                    else:
                        nc.vector.scalar_tensor_tensor(
                            acc_v[:, :D],
                            slot,
                            mask[:, col : col + 1],
                            acc_v[:, :D],
                            mybir.AluOpType.mult,
                            mybir.AluOpType.add,
                        )

            # fold the DVE accumulator (+count column) through PSUM
            nc.tensor.matmul(
                psum_v[:], ones_col[:], acc_v[:], start=True, stop=True
            )

            # ---- phase 2: AllGather partials [257] across the 8 cores ----
            part_sb = small.tile([1, CC], f32)
            nc.vector.tensor_copy(part_sb[:], psum_v[:])
            nc.vector.tensor_add(part_sb[:, :D], part_sb[:, :D], psum_s[:])
            cc_in = dram.tile([CC], f32)
            cc_out = dram.tile([N_CORES * CC], f32)
            nc.sync.dma_start(cc_in[:].rearrange("(o d) -> o d", o=1), part_sb[:])
            nc.gpsimd.collective_compute(
                "AllGather",
                mybir.AluOpType.bypass,
                replica_groups=[list(range(N_CORES))],
                ins=[cc_in.opt()],
                outs=[cc_out.opt()],
            )

            # local reduction of the 8 partials:
            # xsum -> [128, 2, 8] (matvec lhsT layout for free), count -> [1, 8]
            cc_view = cc_out[:].rearrange("(r j) -> j r", r=N_CORES)  # [257, 8]
            xsum8 = small.tile([P, 2 * N_CORES], f32)
            for k in range(2):
                nc.sync.dma_start(
                    xsum8[:, k * N_CORES : (k + 1) * N_CORES],
                    cc_view[k * P : (k + 1) * P, :],
                )
            cnt8 = small.tile([1, N_CORES], f32)
            nc.sync.dma_start(cnt8[:], cc_view[D : D + 1, :])
            xsum_t = small.tile([P, 2], f32)
            nc.vector.reduce_sum(
                xsum_t[:],
                xsum8[:].rearrange("p (k r) -> p k r", k=2),
                axis=mybir.AxisListType.X,
            )
            cnt_sb = small.tile([1, 1], f32)
            nc.vector.reduce_sum(cnt_sb[:], cnt8[:], axis=mybir.AxisListType.X)

            # ---- phase 3: s = xsum @ W.T + count * b ----
            psum_sv = psum_acc.tile([1, D], f32)
            for k in range(2):
                nc.tensor.matmul(
                    psum_sv[:],
                    xsum_t[:, k : k + 1],
                    wt_sb[:, k * D : (k + 1) * D],
                    start=(k == 0),
                    stop=(k == 1),
                )
            s2 = small.tile([1, 2 * D], f32)
            cb = small.tile([1, D], f32)
            nc.vector.tensor_scalar(
                cb[:], b_sb[:], cnt_sb[:], None, mybir.AluOpType.mult
            )
            nc.vector.tensor_add(s2[:, :D], psum_sv[:], cb[:])
            nc.vector.tensor_copy(s2[:, D:], s2[:, :D])

            # ---- phase 4: broadcast s to [128, BC_SLOTS*256], stream out ----
            psum_bc = psum_bc_pool.tile([P, 2 * D], f32)
            nc.tensor.matmul(
                psum_bc[:], ones_row[:], s2[:], start=True, stop=True
            )
            bc_sb = consts.tile([P, BC_SLOTS * D], f32)
            nc.vector.tensor_copy(bc_sb[:, : 2 * D], psum_bc[:])
            nc.vector.tensor_copy(bc_sb[:, 2 * D : 4 * D], bc_sb[:, : 2 * D])
            nc.vector.tensor_copy(bc_sb[:, 4 * D :], bc_sb[:, : 4 * D])

            bc_v = bc_sb[:].rearrange("p (n d) -> p n d", d=D)
            for w in range(0, SLOTS, WCHUNK_SLOTS):
                ns = min(WCHUNK_SLOTS, SLOTS - w)
                nc.sync.dma_start(out_r[:, w : w + ns, :], bc_v[:, :ns, :])

    nc.compile()
    return nc


def _get_compiled():
    global _compiled
    if _compiled is None:
        _compiled = _build()
    return _compiled


def kernel(x, edge_index, node_rankings, W, b):
    x = np.asarray(x, dtype=np.float32)
    W = np.asarray(W, dtype=np.float32)
    b = np.asarray(b, dtype=np.float32)

    xp = np.zeros((NPAD, D), dtype=np.float32)
    xp[:N] = x
    # rankings as f32 (exact for values < 2^24); pad rows get +inf -> mask 0
    rkp = np.full((NPAD,), np.inf, dtype=np.float32)
    rkp[:N] = np.asarray(node_rankings).astype(np.float32)
    wt = np.ascontiguousarray(W.T)

    in_maps = [
        {
            "x": np.ascontiguousarray(xp[i * SHARD : (i + 1) * SHARD]),
            "rk": np.ascontiguousarray(rkp[i * SHARD : (i + 1) * SHARD]),
            "wt": wt,
            "bv": b,
        }
        for i in range(N_CORES)
    ]

    nc = _get_compiled()
    res = run_bass_kernel_spmd(nc, in_maps, core_ids=list(range(N_CORES)))
    full = np.concatenate([res.results[i]["out"] for i in range(N_CORES)], axis=0)
    return np.ascontiguousarray(full[:N])


# revision 9
# speedup vs baseline: 1.3300x; 1.3061x over previous
"""Trainium2 Bass kernel for nn_DenseControl_81793357185325.

Math: out = broadcast_to(s, [N, D]) where
    s = sum_{n: rank[n] <= 100} (x[n] @ W.T + b)
      = (sum_masked x[n]) @ W.T + count * b

Two SPMD launches on 8 NeuronCores (data-parallel over rows, padded to
100352 = 8 * 128 * 98; partition p of a core holds shard rows
[p*98, (p+1)*98)):

  Launch A (per core): stream the x shard [12544, 256] through SBUF in
  14 chunks; per 98-row "slot" accumulate mask-weighted sums — even
  slots on PE (PSUM accumulate, lhsT = mask column), odd slots on DVE
  (scalar_tensor_tensor multiply-add) so the phase is DMA-bound. The
  masked-row count rides as column 256. Output: partial [257].

  Host: concatenates the 8 partials (gather step — no math).

  Launch B (per core): sum the 8 partials (strided load lands xsum in
  the [128, 2] matvec layout), s = xsum @ W.T + count * b on PE,
  broadcast s to [128, 8*256] and stream the output shard back.

No collective: a cross-core rendezvous makes every core pay the SPMD
launch skew (~20us measured); two independent launches don't.
"""

import numpy as np

import concourse.bass as bass
import concourse.tile as tile
from concourse import bacc, mybir
from concourse.bass_utils import run_bass_kernel_spmd

N_CORES = 8
N = 100000
D = 256
K_THRESH = 100.0

P = 128                    # SBUF partitions
SLOTS = 98                 # rows per partition per core
SHARD = P * SLOTS          # 12544 rows per core
NPAD = SHARD * N_CORES     # 100352

CHUNK_SLOTS = 7            # slots per x DMA (7 KiB contiguous per partition)
N_CHUNKS = SLOTS // CHUNK_SLOTS   # 14 read DMAs per core

CC = D + 1                 # partial payload: [xsum(256) | count]

BC_SLOTS = 8               # copies of s in the broadcast tile
WCHUNK_SLOTS = 7           # slots per output DMA -> 14 write DMAs

f32 = mybir.dt.float32

_compiled = None


def _build_a():
    """Launch A: per-core masked partial sums -> [257]."""
    nc = bacc.Bacc(
        "TRN2", target_bir_lowering=False, debug=False, num_devices=N_CORES
    )
    x = nc.dram_tensor("x", [SHARD, D], f32, kind="ExternalInput")
    rk = nc.dram_tensor("rk", [SHARD], f32, kind="ExternalInput")
    part = nc.dram_tensor("part", [CC], f32, kind="ExternalOutput")

    x_r = x.ap().rearrange("(p n) d -> p n d", p=P)  # [128, 98, 256]

    with tile.TileContext(nc) as tc:
        with (
            tc.tile_pool(name="xin", bufs=6) as xin,
            tc.tile_pool(name="consts", bufs=1) as consts,
            tc.tile_pool(name="psum_acc", bufs=1, space="PSUM") as psum_acc,
        ):
            mask = consts.tile([P, SLOTS], f32)
            rk_tile = consts.tile([P, SLOTS], f32)
            nc.sync.dma_start(rk_tile[:], rk.ap().rearrange("(p n) -> p n", p=P))
            nc.vector.tensor_scalar(
                mask[:], rk_tile[:], K_THRESH, None, mybir.AluOpType.is_le
            )

            ones_col = consts.tile([P, 1], f32)
            nc.vector.memset(ones_col[:], 1.0)

            # DVE-side accumulator; col 256 holds the mask row-sums (count)
            acc_v = consts.tile([P, CC], f32)
            nc.vector.memset(acc_v[:], 0.0)
            nc.vector.reduce_sum(
                acc_v[:, D : D + 1], mask[:], axis=mybir.AxisListType.X
            )

            psum_s = psum_acc.tile([1, D], f32)
            psum_v = psum_acc.tile([1, CC], f32)
            for c in range(N_CHUNKS):
                xt = xin.tile([P, CHUNK_SLOTS * D], f32)
                nc.sync.dma_start(
                    xt[:].rearrange("p (n d) -> p n d", d=D),
                    x_r[:, c * CHUNK_SLOTS : (c + 1) * CHUNK_SLOTS, :],
                )
                for s in range(CHUNK_SLOTS):
                    col = c * CHUNK_SLOTS + s
                    slot = xt[:, s * D : (s + 1) * D]
                    if col % 2 == 0:
                        nc.tensor.matmul(
                            psum_s[:],
                            mask[:, col : col + 1],
                            slot,
                            start=(col == 0),
                            stop=(col == SLOTS - 2),
                        )
                    else:
                        nc.vector.scalar_tensor_tensor(
                            acc_v[:, :D],
                            slot,
                            mask[:, col : col + 1],
                            acc_v[:, :D],
                            mybir.AluOpType.mult,
                            mybir.AluOpType.add,
                        )

            # fold the DVE accumulator (+count column) through PSUM
            nc.tensor.matmul(
                psum_v[:], ones_col[:], acc_v[:], start=True, stop=True
            )
            part_sb = consts.tile([1, CC], f32)
            nc.vector.tensor_copy(part_sb[:], psum_v[:])
            nc.vector.tensor_add(part_sb[:, :D], part_sb[:, :D], psum_s[:])
            nc.sync.dma_start(part.ap().rearrange("(o d) -> o d", o=1), part_sb[:])

    nc.compile()
    return nc


def _build_b():
    """Launch B: partials -> s -> broadcast-write the output shard."""
    nc = bacc.Bacc(
        "TRN2", target_bir_lowering=False, debug=False, num_devices=N_CORES
    )
    parts = nc.dram_tensor("parts", [N_CORES * CC], f32, kind="ExternalInput")
    wt = nc.dram_tensor("wt", [D, D], f32, kind="ExternalInput")
    bv = nc.dram_tensor("bv", [D], f32, kind="ExternalInput")
    out = nc.dram_tensor("out", [SHARD, D], f32, kind="ExternalOutput")

    out_r = out.ap().rearrange("(p n) d -> p n d", p=P)  # [128, 98, 256]

    with tile.TileContext(nc) as tc:
        with (
            tc.tile_pool(name="consts", bufs=1) as consts,
            tc.tile_pool(name="psum", bufs=1, space="PSUM") as psum_pool,
        ):
            wt_sb = consts.tile([P, 2 * D], f32)
            for k in range(2):
                nc.sync.dma_start(
                    wt_sb[:, k * D : (k + 1) * D], wt[k * P : (k + 1) * P, :]
                )
            b_sb = consts.tile([1, D], f32)
            nc.sync.dma_start(b_sb[:], bv.ap().rearrange("(o d) -> o d", o=1))
            ones_row = consts.tile([1, P], f32)
            nc.vector.memset(ones_row[:], 1.0)

            # sum the 8 partials; strided load -> matvec lhsT layout
            pv = parts[:].rearrange("(r j) -> j r", r=N_CORES)  # [257, 8]
            xsum8 = consts.tile([P, 2 * N_CORES], f32)
            for k in range(2):
                nc.sync.dma_start(
                    xsum8[:, k * N_CORES : (k + 1) * N_CORES],
                    pv[k * P : (k + 1) * P, :],
                )
            cnt8 = consts.tile([1, N_CORES], f32)
            nc.sync.dma_start(cnt8[:], pv[D : D + 1, :])

            xsum_t = consts.tile([P, 2], f32)
            nc.vector.reduce_sum(
                xsum_t[:],
                xsum8[:].rearrange("p (k r) -> p k r", k=2),
                axis=mybir.AxisListType.X,
            )
            cnt_sb = consts.tile([1, 1], f32)
            nc.vector.reduce_sum(cnt_sb[:], cnt8[:], axis=mybir.AxisListType.X)

            # s = xsum @ W.T + count * b
            psum_sv = psum_pool.tile([1, D], f32)
            for k in range(2):
                nc.tensor.matmul(
                    psum_sv[:],
                    xsum_t[:, k : k + 1],
                    wt_sb[:, k * D : (k + 1) * D],
                    start=(k == 0),
                    stop=(k == 1),
                )
            s2 = consts.tile([1, 2 * D], f32)
            cb = consts.tile([1, D], f32)
            nc.vector.tensor_scalar(
                cb[:], b_sb[:], cnt_sb[:], None, mybir.AluOpType.mult
            )
            nc.vector.tensor_add(s2[:, :D], psum_sv[:], cb[:])
            nc.vector.tensor_copy(s2[:, D:], s2[:, :D])

            # broadcast s to [128, BC_SLOTS*256] and stream out
            psum_bc = psum_pool.tile([P, 2 * D], f32)
            nc.tensor.matmul(psum_bc[:], ones_row[:], s2[:], start=True, stop=True)
            bc_sb = consts.tile([P, BC_SLOTS * D], f32)
            nc.vector.tensor_copy(bc_sb[:, : 2 * D], psum_bc[:])
            nc.vector.tensor_copy(bc_sb[:, 2 * D : 4 * D], bc_sb[:, : 2 * D])
            nc.vector.tensor_copy(bc_sb[:, 4 * D :], bc_sb[:, : 4 * D])

            bc_v = bc_sb[:].rearrange("p (n d) -> p n d", d=D)
            for w in range(0, SLOTS, WCHUNK_SLOTS):
                ns = min(WCHUNK_SLOTS, SLOTS - w)
                nc.sync.dma_start(out_r[:, w : w + ns, :], bc_v[:, :ns, :])

    nc.compile()
    return nc


def _get_compiled():
    global _compiled
    if _compiled is None:
        _compiled = (_build_a(), _build_b())
    return _compiled


def _run(inputs_x, rankings, W, b, trace=False, trace_cores=None):
    """Returns (full_out, exec_a_ns, exec_b_ns)."""
    xp = np.zeros((NPAD, D), dtype=np.float32)
    xp[:N] = inputs_x
    rkp = np.full((NPAD,), np.inf, dtype=np.float32)
    rkp[:N] = np.asarray(rankings).astype(np.float32)
    wt = np.ascontiguousarray(np.asarray(W, dtype=np.float32).T)
    bv = np.ascontiguousarray(np.asarray(b, dtype=np.float32))

    nc_a, nc_b = _get_compiled()
    kw = dict(trace=trace, trace_cores=trace_cores) if trace else {}

    in_a = [
        {
            "x": np.ascontiguousarray(xp[i * SHARD : (i + 1) * SHARD]),
            "rk": np.ascontiguousarray(rkp[i * SHARD : (i + 1) * SHARD]),
        }
        for i in range(N_CORES)
    ]
    res_a = run_bass_kernel_spmd(nc_a, in_a, core_ids=list(range(N_CORES)), **kw)

    parts = np.concatenate(
        [res_a.results[i]["part"] for i in range(N_CORES)], axis=0
    )
    in_b = [{"parts": parts, "wt": wt, "bv": bv} for _ in range(N_CORES)]
    res_b = run_bass_kernel_spmd(nc_b, in_b, core_ids=list(range(N_CORES)), **kw)

    full = np.concatenate(
        [res_b.results[i]["out"] for i in range(N_CORES)], axis=0
    )
    return (
        np.ascontiguousarray(full[:N]),
        res_a.exec_time_ns,
        res_b.exec_time_ns,
    )


def kernel(x, edge_index, node_rankings, W, b):
    x = np.asarray(x, dtype=np.float32)
    out, _, _ = _run(x, node_rankings, W, b)
    return out
